# revision 49
# baseline (speedup 1.0000x reference)
import numpy as np

import concourse.bass as bass
import concourse.bacc as bacc
import concourse.mybir as mybir
import concourse.tile as tile
from concourse.bass_utils import run_bass_kernel_spmd

F32 = mybir.dt.float32
F16 = mybir.dt.float16
AX = mybir.AxisListType
ADD = mybir.AluOpType.add
SUB = mybir.AluOpType.subtract

MODE = "f16x3"

N_CORES = 8
B_FULL, H = 16384, 2048
B = B_FULL // N_CORES
P = 128
NT = B // P
NCH = H // P
GC = 4
NG = NCH // GC
J = 13


def _build_program(mode):
    split = mode == "f16x3"
    mmdt = F16
    CW = 2 * J if split else J

    nc = bacc.Bacc("TRN2", target_bir_lowering=False, debug=False,
                   num_devices=N_CORES)
    x_d = nc.dram_tensor("x", [B, H], F32, kind="ExternalInput")
    w_d = nc.dram_tensor("w", [H, CW], mmdt, kind="ExternalInput")
    b_d = nc.dram_tensor("b", [1, CW], mmdt, kind="ExternalInput")
    e_d = nc.dram_tensor("ident", [P, P], F32, kind="ExternalInput")
    y_d = nc.dram_tensor("y", [B, 5], F32, kind="ExternalOutput")

    with tile.TileContext(nc) as tc:
        with (
            tc.tile_pool(name="const", bufs=1) as cpool,
            tc.tile_pool(name="xin", bufs=6) as xin_pool,
            tc.tile_pool(name="xgp", bufs=2) as xg_pool,
            tc.tile_pool(name="xt", bufs=4) as xt_pool,
            tc.tile_pool(name="tp", bufs=6, space="PSUM") as tp_pool,
            tc.tile_pool(name="acc", bufs=2, space="PSUM") as acc_pool,
            tc.tile_pool(name="work", bufs=1) as wpool,
            tc.tile_pool(name="tailp", bufs=2) as tpool,
        ):
            id_sb = cpool.tile([P, P], F32)
            nc.sync.dma_start(id_sb[:], e_d.ap())
            b_sb = cpool.tile([1, CW], mmdt)
            nc.sync.dma_start(b_sb[:], b_d.ap())
            w_sb = cpool.tile([P, NCH, CW], mmdt)
            nc.gpsimd.dma_start(w_sb[:],
                                w_d.ap().rearrange("(c p) j -> p c j", p=P))
            ones_sb = cpool.tile([1, P], mmdt)
            nc.gpsimd.memset(ones_sb[:], 1.0)

            NTH = NT // 2
            all_st = [wpool.tile([P, NTH, J], F32, tag=f"st{h}",
                                 name=f"all_st{h}")
                      for h in range(2)]

            y3 = y_d.ap().rearrange("(t p) j -> p t j", p=P)

            def tail(h):
                st = all_st[h][:]
                e_all = tpool.tile([P, NTH, J], F32, tag="e_all")
                nc.scalar.activation(e_all[:], st,
                                     mybir.ActivationFunctionType.Exp)
                EG = e_all[:, :, 0:4]
                EP = e_all[:, :, 4:8]
                EA = e_all[:, :, 8:13]

                sg = tpool.tile([P, NTH], F32, tag="sg")
                nc.vector.tensor_reduce(sg[:], EG, axis=AX.X, op=ADD)
                sp = tpool.tile([P, NTH], F32, tag="sp")
                nc.vector.tensor_reduce(sp[:], EP, axis=AX.X, op=ADD)
                ss = tpool.tile([P, NTH], F32, tag="ss")
                nc.vector.tensor_mul(ss[:], sp[:], sg[:])

                tmp4 = tpool.tile([P, NTH, 4], F32, tag="tmp4")
                nc.vector.tensor_mul(tmp4[:], EP, EG)
                u3 = tpool.tile([P, NTH, 3], F32, tag="u3")
                nc.vector.tensor_reduce(u3[:, :, 0], tmp4[:], axis=AX.X,
                                        op=ADD)

                tmp2 = tpool.tile([P, NTH, 2], F32, tag="tmp2")
                nc.vector.tensor_mul(tmp2[:], e_all[:, :, 4:8:2],
                                     e_all[:, :, 1:4:2])
                nc.vector.tensor_reduce(u3[:, :, 1], tmp2[:], axis=AX.X,
                                        op=ADD)

                tmp2b = tpool.tile([P, NTH, 2], F32, tag="tmp2b")
                nc.vector.tensor_mul(tmp2b[:], e_all[:, :, 5:8:2],
                                     e_all[:, :, 0:3:2])
                nc.vector.tensor_reduce(u3[:, :, 2], tmp2b[:], axis=AX.X,
                                        op=ADD)

                V = tpool.tile([P, NTH, 5], F32, tag="V")
                nc.vector.tensor_sub(V[:, :, 0:3],
                                     ss[:].broadcast_to([P, NTH, 3]), u3[:])
                nc.vector.tensor_copy(V[:, :, 3:5],
                                      ss[:].broadcast_to([P, NTH, 2]))

                tj = tpool.tile([P, NTH, 5], F32, tag="tj")
                nc.vector.tensor_mul(tj[:], EA, V[:])
                s5 = tpool.tile([P, NTH], F32, tag="s5")
                nc.vector.tensor_reduce(s5[:], tj[:], axis=AX.X, op=ADD)
                r5 = tpool.tile([P, NTH], F32, tag="r5")
                nc.vector.reciprocal(r5[:], s5[:])

                out_sb = tpool.tile([P, NTH, 5], F32, tag="out_sb")
                nc.vector.tensor_mul(out_sb[:], tj[:],
                                     r5[:].broadcast_to([P, NTH, 5]))
                nc.scalar.dma_start(y3[:, h * NTH:(h + 1) * NTH, :],
                                    out_sb[:])

            def fold(t, acc):
                if split:
                    tlo = xt_pool.tile([P, J], F32, tag="tlo")
                    nc.scalar.copy(tlo[:], acc[:, J:2 * J])
                    nc.vector.tensor_add(
                        all_st[t // NTH][:, t % NTH, :], acc[:, 0:J], tlo[:])
                else:
                    nc.scalar.copy(all_st[t // NTH][:, t % NTH, :], acc[:])
                if t % NTH == NTH - 1:
                    tail(t // NTH)

            def emit_matmuls(t, g, acc, hi8, lo8):
                for k in range(GC):
                    c = GC * g + k
                    last = c == NCH - 1
                    sl = slice(k * P, (k + 1) * P)
                    if split:
                        nc.tensor.matmul(acc[:, 0:J], lo8[:, sl],
                                         w_sb[:, c, 0:J],
                                         start=False, stop=False,
                                         skip_group_check=True)
                    nc.tensor.matmul(acc[:], hi8[:, sl], w_sb[:, c, :],
                                     start=False, stop=last,
                                     skip_group_check=True)
                if g == NG - 1:
                    fold(t, acc)

            GP_TILES = ()
            xg_tiles = {}
            for t in GP_TILES:
                xg = xg_pool.tile([P, H], F32, tag="xg", name=f"xg{t}")
                nc.gpsimd.dma_start(xg[:], x_d.ap()[t * P:(t + 1) * P, :])
                xg_tiles[t] = xg

            pend = None

            for t in range(NT):
                if t in xg_tiles:
                    xt_full = xg_tiles[t]

                    def chunk(c, xt_full=xt_full):
                        return xt_full[:, c * P:(c + 1) * P]
                else:
                    xq = []
                    for q in range(4):
                        xqt = xin_pool.tile([P, H // 4], F32, tag=f"xs{q}",
                                            name=f"xs{t}_{q}")
                        nc.sync.dma_start(
                            xqt[:],
                            x_d.ap()[t * P:(t + 1) * P,
                                     q * (H // 4):(q + 1) * (H // 4)])
                        xq.append(xqt)

                    def chunk(c, xq=xq):
                        return xq[c // GC][:, (c % GC) * P:(c % GC + 1) * P]

                acc = acc_pool.tile([P, CW], F32)
                nc.tensor.matmul(acc[:], ones_sb[:], b_sb[:],
                                 start=True, stop=False, skip_group_check=True)
                for g in range(NG):
                    tp = tp_pool.tile([P, GC * P], F32)
                    for k in range(GC):
                        c = GC * g + k
                        nc.tensor.transpose(
                            tp[:, k * P:(k + 1) * P],
                            chunk(c),
                            id_sb[:])
                    hi8 = xt_pool.tile([P, GC * P], mmdt, tag="hi")
                    if split:
                        nc.scalar.copy(hi8[:], tp[:])
                        lo8 = xt_pool.tile([P, GC * P], F16, tag="lo")
                        nc.vector.tensor_tensor(lo8[:], tp[:], hi8[:], op=SUB)
                    else:
                        if g % 2 == 0:
                            nc.scalar.copy(hi8[:], tp[:])
                        else:
                            nc.vector.tensor_copy(hi8[:], tp[:])
                        lo8 = None
                    if pend is not None:
                        emit_matmuls(*pend)
                    pend = (t, g, acc, hi8, lo8)
            emit_matmuls(*pend)

    nc.compile()
    return nc


_NC_CACHE = {}


def _get_program(mode=MODE):
    if mode not in _NC_CACHE:
        _NC_CACHE[mode] = _build_program(mode)
    return _NC_CACHE[mode]


def _prep_in_maps(x, Wg, bg, Wp, bp, Wa, ba, mode=MODE):
    x = np.ascontiguousarray(np.asarray(x, dtype=np.float32))
    W = np.concatenate([np.asarray(Wg), np.asarray(Wp), np.asarray(Wa)],
                       axis=1).astype(np.float32)
    bvec = np.concatenate([np.asarray(bg), np.asarray(bp), np.asarray(ba)]
                          ).astype(np.float32).reshape(1, J)
    ident = np.eye(P, dtype=np.float32)
    if mode == "f16x3":
        Whi = W.astype(np.float16)
        Wlo = (W - Whi.astype(np.float32)).astype(np.float16)
        w_dev = np.concatenate([Whi, Wlo], axis=1)
        b_dev = np.concatenate([bvec, np.zeros_like(bvec)],
                               axis=1).astype(np.float16)
    else:
        w_dev = W.astype(np.float16)
        b_dev = bvec.astype(np.float16)
    in_maps = []
    for i in range(N_CORES):
        in_maps.append({
            "x": x[i * B:(i + 1) * B],
            "w": w_dev,
            "b": b_dev,
            "ident": ident,
        })
    return in_maps


def kernel(x, Wg, bg, Wp, bp, Wa, ba):
    in_maps = _prep_in_maps(x, Wg, bg, Wp, bp, Wa, ba)
    nc = _get_program()
    res = run_bass_kernel_spmd(nc, in_maps, core_ids=list(range(N_CORES)))
    return np.concatenate([res.results[i]["y"] for i in range(N_CORES)],
                          axis=0)
